# revision 1
# baseline (speedup 1.0000x reference)
"""Bass/Trainium2 kernel for nn_Net_27882927686181 (gnn_message_passing).

Computation: v0 = sigmoid(x + 1); 12 layers of
    v <- sigmoid(einsum('bmk,mk->bm', v[:, idx[l]], W[l]) + b[l])
with B=1024, M=2048, K=32, L=12.

Strategy (8 NeuronCores, SPMD):
  - Node-sharded compute: core c owns nodes [256c, 256c+256) of every layer,
    for the FULL batch of 1024.
  - Per layer, the layer's value table v^T [2048 nodes, 1024 batch] lives in
    DRAM on every core. Each core issues 64 indirect-DMA gathers (dynamic
    vector-offset DGE); each gather pulls 128 table rows (one per SBUF
    partition) = one "group" of 4 nodes x 32 fan-in, full-batch wide.
  - PE computes the fused multiply-reduce: stationary weights are a
    [128 x 128] block-diagonal expansion of W for the group (host-built), so
    out_psum[4g'+j, b] += sum_k gathered[32j+k, b] * W[node, k]. 32 groups
    accumulate into one [128, 1024] PSUM block = 128 output nodes.
  - ACT applies sigmoid(psum + bias) with per-partition (= per-node) bias.
  - The 256-node shard is stored to DRAM and AllGathered across the 8 cores
    to form the next layer's table.
"""

import os
import numpy as np

B, M, K, L = 1024, 2048, 32, 12
N_CORES = 8
NODES_PER_CORE = M // N_CORES          # 256
GROUPS = NODES_PER_CORE // 4           # 64 groups of 4 nodes (128 gather rows)
GROUPS_PER_BLOCK = 32                  # 32 groups -> 128-node PSUM block

_cache = {}


def _patch_walrus():
    """Enable vector-dynamic-offset DGE lowering in the minimal walrus pipeline."""
    import concourse.bass_utils as bu
    if getattr(bu, "_ant_dge_patched", False):
        return
    orig = bu.run_command
    dge = ("--dge-levels=io,spill_reload,scalar_dynamic_offset,"
           "vector_dynamic_offsets,dst_reduce,transpose")

    def patched(argv, **kwargs):
        if argv and "walrus_driver" in str(argv[0]):
            argv = list(argv)
            for i, a in enumerate(argv):
                if a == "--pass":
                    passes = argv[i + 1].split(",")
                    for p in ("expand_inst_late", "coloring_allocator_reg"):
                        if p not in passes:
                            passes.insert(passes.index("codegen"), p)
                    argv[i + 1] = ",".join(passes)
                    break
            argv.append(dge)
        return orig(argv, **kwargs)

    bu.run_command = patched
    bu._ant_dge_patched = True


def _split_multi_waits(nc, max_waits=1):
    """walrus codegen rejects >max sem waits per instruction; split onto NOPs."""
    import bass_rust
    from concourse import mybir
    n = 0
    for f in nc.m.functions:
        for blk in f.blocks:
            il = blk.instructions
            i = 0
            while i < len(il):
                inst = il[i]
                si = inst.sync_info
                if si is not None and len(si.on_wait) > max_waits:
                    waits = list(si.on_wait)
                    si.on_wait = waits[:max_waits]
                    extra = waits[max_waits:]
                    pos = i
                    for j in range(0, len(extra), max_waits):
                        nop = mybir.InstNoOp(name=f"Wsplit{n}-{j}", ins=[], outs=[])
                        nop.engine = inst.engine
                        nop.sync_info = bass_rust.SyncInfo(
                            on_wait=extra[j:j + max_waits], on_update=[])
                        il.insert(pos, nop)
                        pos += 1
                        i += 1
                    n += 1
                i += 1
    return n


def _build():
    import concourse.bass as bass
    import concourse.tile as tile
    from concourse import mybir
    from concourse.bass import IndirectOffsetOnAxis
    from concourse.tile import add_dep_helper

    _patch_walrus()

    f32 = mybir.dt.float32
    f16 = mybir.dt.float16
    nc = bass.Bass("TRN2", target_bir_lowering=False, debug=False,
                   num_devices=N_CORES)

    xT_d = nc.dram_tensor("xT", [M, B], f32, kind="ExternalInput").ap()
    idx_d = nc.dram_tensor("idxp", [128, L * GROUPS], mybir.dt.int32,
                           kind="ExternalInput").ap()
    wst_d = nc.dram_tensor("wst", [128, L * GROUPS * 128], f16,
                           kind="ExternalInput").ap()
    b_d = nc.dram_tensor("bp", [128, L * 2], f32, kind="ExternalInput").ap()
    out_d = nc.dram_tensor("out", [NODES_PER_CORE, B], f32,
                           kind="ExternalOutput").ap()

    if os.environ.get("K_LOCALVT"):
        vt = [nc.dram_tensor(f"vt{i}", [M, B], f16).ap() for i in range(2)]
    else:
        vt = [nc.dram_tensor(f"vt{i}", [M, B], f16, addr_space="Shared").ap()
              for i in range(2)]
    shard = nc.dram_tensor("shard", [NODES_PER_CORE, B], f16).ap()

    with tile.TileContext(nc) as tc:
        with tc.tile_pool(name="const", bufs=1) as cpool, \
             tc.tile_pool(name="wst", bufs=2) as wpool, \
             tc.tile_pool(name="gath", bufs=4) as gpool, \
             tc.tile_pool(name="sig", bufs=2) as spool, \
             tc.tile_pool(name="init", bufs=3) as ipool, \
             tc.tile_pool(name="psum", bufs=2, space="PSUM") as ppool:

            idx_sb = cpool.tile([128, L * GROUPS], mybir.dt.int32)
            nc.sync.dma_start(idx_sb[:], idx_d[:])
            b_sb = cpool.tile([128, L * 2], f32)
            nc.sync.dma_start(b_sb[:], b_d[:])

            # ---- init: vt[0] = sigmoid(xT + 1) ----
            init_stores = []
            for t in range(M // 128):
                xt = ipool.tile([128, B], f32, tag="xt")
                nc.sync.dma_start(xt[:], xT_d[128 * t:128 * (t + 1), :])
                s = ipool.tile([128, B], f16, tag="s0")
                nc.scalar.activation(s[:], xt[:],
                                     mybir.ActivationFunctionType.Sigmoid,
                                     bias=1.0, scale=1.0)
                st = nc.sync.dma_start(vt[0][128 * t:128 * (t + 1), :], s[:])
                init_stores.append(st.ins)

            prev_table_writers = init_stores      # insts that wrote current table
            layer_gathers_prev = None             # gathers of previous layer

            for l in range(L):
                src = vt[l % 2]
                gathers = []
                shard_stores = []

                # stream the layer's stationary weights in 2 half-layer chunks
                wst_tiles = []
                for h in range(2):
                    wt = wpool.tile([128, GROUPS_PER_BLOCK * 128], f16, tag="w")
                    off = (l * GROUPS + h * GROUPS_PER_BLOCK) * 128
                    nc.sync.dma_start(
                        wt[:], wst_d[:, off:off + GROUPS_PER_BLOCK * 128])
                    wst_tiles.append(wt)

                for blk in range(2):
                    _nomm = bool(os.environ.get("K_NOMM"))
                    psum = None if _nomm else ppool.tile([128, B], f32, tag="ps")
                    wt = wst_tiles[blk]
                    for gl in range(GROUPS_PER_BLOCK):
                        g = blk * GROUPS_PER_BLOCK + gl
                        gt = gpool.tile([128, B], f16, tag="g")
                        ii = nc.gpsimd.indirect_dma_start(
                            gt[:].bitcast(mybir.dt.uint32), None,
                            src.bitcast(mybir.dt.uint32),
                            IndirectOffsetOnAxis(
                                ap=idx_sb[:, l * GROUPS + g:l * GROUPS + g + 1],
                                axis=0))
                        gathers.append(ii.ins)
                        # table-ready dependency
                        for w in prev_table_writers:
                            add_dep_helper(ii.ins, w, sync=True,
                                           reason="table ready")
                        for h in ([] if os.environ.get("K_NOMM") else range(2)):
                            nc.tensor.matmul(
                                out=psum[:, h * 512:(h + 1) * 512],
                                lhsT=wt[:, gl * 128:(gl + 1) * 128],
                                rhs=gt[:, h * 512:(h + 1) * 512],
                                start=(gl == 0),
                                stop=(gl == GROUPS_PER_BLOCK - 1))

                    if _nomm:
                        continue
                    sig = spool.tile([128, B],
                                     f32 if l == L - 1 else f16,
                                     tag="sig32" if l == L - 1 else "sig")
                    nc.scalar.activation(sig[:], psum[:],
                                         mybir.ActivationFunctionType.Sigmoid,
                                         bias=b_sb[:, l * 2 + blk:l * 2 + blk + 1],
                                         scale=1.0)
                    if l == L - 1:
                        nc.sync.dma_start(
                            out_d[128 * blk:128 * (blk + 1), :], sig[:])
                    else:
                        st = nc.sync.dma_start(
                            shard[128 * blk:128 * (blk + 1), :], sig[:])
                        shard_stores.append(st.ins)

                if l < L - 1:
                    dst = vt[(l + 1) % 2]
                    if os.environ.get("K_NOCOLL"):
                        cc = nc.sync.dma_start(dst[0:NODES_PER_CORE, :], shard[:])
                    else:
                        cc = nc.gpsimd.collective_compute(
                            "AllGather", mybir.AluOpType.bypass,
                            replica_groups=[list(range(N_CORES))],
                            ins=[shard[:]], outs=[dst[:]])
                    for st in shard_stores:
                        add_dep_helper(cc.ins, st, sync=True,
                                       reason="shard stored")
                    # WAR: don't overwrite the table layer l-1 read
                    if layer_gathers_prev:
                        for gi in layer_gathers_prev:
                            add_dep_helper(cc.ins, gi, sync=True,
                                           reason="table WAR")
                    prev_table_writers = [cc.ins]
                layer_gathers_prev = gathers

    _split_multi_waits(nc, max_waits=1)
    return nc


def _get_runner():
    if "runner" in _cache:
        return _cache["runner"]
    import jax
    import concourse.mybir as mybir
    import concourse.bass2jax as bass2jax
    from concourse.bass2jax import _bass_exec_p, install_neuronx_cc_hook
    from jax.sharding import Mesh, PartitionSpec
    from jax.experimental.shard_map import shard_map

    nc = _build()
    install_neuronx_cc_hook()

    partition_name = nc.partition_id_tensor.name if nc.partition_id_tensor else None
    in_names, out_names, out_avals, zero_outs = [], [], [], []
    for alloc in nc.m.functions[0].allocations:
        if not isinstance(alloc, mybir.MemoryLocationSet):
            continue
        name = alloc.memorylocations[0].name
        if alloc.kind == "ExternalInput":
            if name != partition_name:
                in_names.append(name)
        elif alloc.kind == "ExternalOutput":
            shape = tuple(alloc.tensor_shape)
            dtype = mybir.dt.np(alloc.dtype)
            out_names.append(name)
            out_avals.append(jax.core.ShapedArray(shape, dtype))
            zero_outs.append(np.zeros(shape, dtype))
    n_params = len(in_names)
    all_in = in_names + out_names
    if partition_name is not None:
        all_in.append(partition_name)

    def _body(*args):
        operands = list(args)
        if partition_name is not None:
            operands.append(bass2jax.partition_id_tensor())
        return tuple(_bass_exec_p.bind(
            *operands,
            out_avals=tuple(out_avals),
            in_names=tuple(all_in),
            out_names=tuple(out_names),
            lowering_input_output_aliases=(),
            sim_require_finite=True,
            sim_require_nnan=True,
            nc=nc))

    devices = jax.devices()[:N_CORES]
    mesh = Mesh(np.asarray(devices), ("core",))
    in_specs = (PartitionSpec("core"),) * (n_params + len(out_names))
    out_specs = (PartitionSpec("core"),) * len(out_names)
    f = jax.jit(shard_map(_body, mesh=mesh, in_specs=in_specs,
                          out_specs=out_specs, check_rep=False),
                keep_unused=True)
    _cache["runner"] = (f, in_names, out_names, zero_outs)
    return _cache["runner"]


def _prep_inputs(x, idx, W, b):
    """Host-side layout prep (sharding + tiling only)."""
    xT = np.ascontiguousarray(x.T.astype(np.float32))           # [M, B]

    # idx_prep[c][p=32j+k, l*GROUPS+g] = idx[l, 256c+4g+j, k]
    idx_r = idx.reshape(L, N_CORES, GROUPS, 4, K)               # l c g j k
    idx_p = np.ascontiguousarray(
        idx_r.transpose(1, 3, 4, 0, 2)                          # c j k l g
        .reshape(N_CORES, 128, L * GROUPS)).astype(np.int32)

    # stationary block-diag weights:
    # wst[c][p, ((l*GROUPS+g)*128) + col] with nonzero at p=32j+k, col=4*(g%32)+j
    W_r = W.reshape(L, N_CORES, GROUPS, 4, K).astype(np.float32)  # l c g j k
    wst = np.zeros((N_CORES, 128, L * GROUPS, 128), dtype=np.float32)
    jj, kk = np.meshgrid(np.arange(4), np.arange(K), indexing="ij")
    p_rows = (32 * jj + kk).ravel()                              # [128]
    for c in range(N_CORES):
        for l in range(L):
            for g in range(GROUPS):
                cols = (4 * (g % GROUPS_PER_BLOCK) + jj).ravel()
                wst[c, p_rows, l * GROUPS + g, cols] = W_r[l, c, g].ravel()
    wst = wst.reshape(N_CORES, 128, L * GROUPS * 128).astype(np.float16)

    # bias blocks: b_prep[c][p, l*2+blk] = b[l, 256c+128blk+p]
    b_r = b.reshape(L, N_CORES, 2, 128).astype(np.float32)       # l c blk p
    b_p = np.ascontiguousarray(b_r.transpose(1, 3, 0, 2)
                               .reshape(N_CORES, 128, L * 2))

    per_core = []
    for c in range(N_CORES):
        per_core.append({
            "xT": xT,
            "idxp": idx_p[c],
            "wst": wst[c],
            "bp": b_p[c],
        })
    return per_core


def kernel(x, idx, W, b):
    import jax
    x = np.asarray(x, dtype=np.float32)
    idx = np.asarray(idx, dtype=np.int32)
    W = np.asarray(W, dtype=np.float32)
    b = np.asarray(b, dtype=np.float32)

    f, in_names, out_names, zero_outs = _get_runner()
    per_core = _prep_inputs(x, idx, W, b)

    args = []
    for n in in_names:
        cat = np.concatenate([per_core[c][n] for c in range(N_CORES)], axis=0)
        args.append(jax.device_put(cat))
    for z in zero_outs:
        args.append(jax.device_put(np.concatenate([z] * N_CORES, axis=0)))

    outs = f(*args)
    jax.block_until_ready(outs)
    full = np.asarray(outs[0])                                   # [8*256, B]
    return np.ascontiguousarray(full.T)                          # [B, M]


if __name__ == "__main__":
    rng = np.random.default_rng(0)
    x = rng.standard_normal((B, M)).astype(np.float32)
    idx = rng.integers(0, M, size=(L, M, K)).astype(np.int32)
    W = rng.standard_normal((L, M, K)).astype(np.float32)
    b = rng.standard_normal((L, M)).astype(np.float32)
    out = kernel(x=x, idx=idx, W=W, b=b)
    # numpy reference
    v = 1.0 / (1.0 + np.exp(-(x + 1.0)))
    for l in range(L):
        g = v[:, idx[l]]                       # [B, M, K]
        v = 1.0 / (1.0 + np.exp(-(np.einsum('bmk,mk->bm', g, W[l]) + b[l])))
    err = np.abs(out - v).max() / max(np.abs(v).max(), 1e-9)
    print("rel err vs numpy:", err)

